# revision 15
# baseline (speedup 1.0000x reference)
"""ColBERT MaxSim contrastive loss on 8 Trainium2 NeuronCores.

scores[b, c] = (1/q_len[b]) * sum_n max_s <q[b, n, :], d[c, s, :]>
loss = CE(scores / T, labels=arange(B)), mean reduction.

Sharding: data-parallel over the *doc* batch dim (columns of the score
matrix). Each core holds the full query set (1 MB) plus its 8-doc shard
(4 MB) instead of the all-gathered 32 MB doc tensor, computes its
(B_global, B_local) = (64, 8) score block fully on device (fp16 matmuls
at full PE rate + split max-reduction), and the host performs the final
gather + tiny 64x64 CE reduction (the same "host sums the partials"
tail as the standard contrastive sharding).

Device pipeline per core:
  1. DMA q (64,32,128) f32 and d_shard (8,1024,128) f32 into SBUF in
     token-major layout (per doc-pair, so the main loop starts while
     later pairs still load), cast to fp16.
  2. One batched xbar DMA-transpose per tensor/pair flips 128x128
     blocks into [D, token] layout (the PE contracts over partitions).
  3. For each (query group g of 4 queries, doc pair): four fp16
     matmuls -> one [128, 2048] PSUM tile (two docs x 1024 tokens).
  4. Max-reduce consumption, split by measured engine rates (PSUM has
     one DVE read port; DVE reduce-from-PSUM ~1.5 cyc/elem, ACT copy
     ~1 cyc/elem):
       path M: DVE reduce_max directly on PSUM -> maxes columns.
       path B: ACT copies the PSUM tile to fp16 SBUF; DVE runs a 4x
               batched reduce_max over the staged fp16.
     ~28% of sets go to M, the rest to B, balancing DVE and ACT.
  5. A tiny selector matmul sums the 32 token-maxes per query:
     out[4, 128] = sel.T @ maxes.
Host: out blocks -> scores (64, 64) -> q_len scaling -> CE loss.
"""

import json

import numpy as np

import concourse.bass as bass
import concourse.mybir as mybir
import concourse.tile as tile
from concourse.bass_utils import run_bass_kernel_spmd

B = 64          # queries (= docs, contrastive batch)
NQ = 32         # tokens per query
ND = 1024       # tokens per doc
D = 128         # embedding dim
NCORES = 8
CL = B // NCORES  # docs per core
TEMPERATURE = 0.02
NORMALIZE_SCORES = True

F32 = mybir.dt.float32
F16 = mybir.dt.float16

NG = (B * NQ) // 128        # 16 query groups of 4 queries
NPAIR = CL // 2             # 4 doc pairs per core
NSETS = NG * CL             # 128 (query group, doc) sets

# Every g: doc pair 0 -> path M; every M_EXTRA_PERIOD-th g additionally
# sends pair 1 to M. Tuned so DVE ~= ACT busy time.
M_EXTRA_PERIOD = 8


def _split_waits_json(bir_bytes: bytes) -> bytes:
    """Walrus in this toolchain rejects >1 sem-wait per instruction on the
    Tile end-of-kernel drain; split extra waits onto preceding Drains."""
    bir = json.loads(bir_bytes)
    for f in bir["functions"]:
        for blk in f["blocks"]:
            fixed = []
            for ins in blk["instructions"]:
                si = ins.get("sync_info") or {}
                waits = si.get("on_wait") or []
                if len(waits) > 1:
                    for i, w in enumerate(waits[:-1]):
                        fixed.append({
                            "debug": ins.get("debug", 0),
                            "engine": ins["engine"],
                            "ins": [],
                            "is_reset_sema": False,
                            "name": f'{ins["name"]}-wsplit{i}',
                            "opcode": "Drain",
                            "outs": [],
                            "sync_info": {"on_update": [], "on_wait": [w]},
                        })
                    si["on_wait"] = waits[-1:]
                    ins["sync_info"] = si
                fixed.append(ins)
            blk["instructions"] = fixed
    return json.dumps(bir).encode()


def _patch_nc(nc):
    orig = nc.to_json_bytes

    def patched(*a, **k):
        return _split_waits_json(orig(*a, **k))

    nc.to_json_bytes = patched
    return nc


def build_nc(m_extra_period=None):
    """Build the per-core Bass program (SPMD: every core runs this; only
    the data in its "d" shard differs)."""
    mper = m_extra_period or M_EXTRA_PERIOD
    nc = bass.Bass("TRN2", target_bir_lowering=False, debug=False,
                   num_devices=NCORES)
    q_dram = nc.dram_tensor("q", [B, NQ, D], F32, kind="ExternalInput").ap()
    d_dram = nc.dram_tensor("d", [CL, ND, D], F32, kind="ExternalInput").ap()
    out_dram = nc.dram_tensor("out", [4, NSETS], F32, kind="ExternalOutput").ap()

    NQTOK = B * NQ          # 2048 query tokens

    with tile.TileContext(nc) as tc:
        with (
            tc.tile_pool(name="prep", bufs=1) as prep,
            tc.tile_pool(name="dload", bufs=2) as dload_pool,
            tc.tile_pool(name="stgb", bufs=2) as stgb_pool,
            tc.tile_pool(name="mm", bufs=2, space="PSUM") as psum_pool,
        ):
            # ---- prologue: load + cast + transpose ----
            q_nat = prep.tile([128, NQTOK], F32)
            # partition p = (b%4)*32 + n, free = (b//4)*128 + d
            nc.sync.dma_start(
                q_nat[:].rearrange("p (t d) -> p t d", t=NQTOK // 128),
                q_dram.rearrange("(t bb) n d -> (bb n) t d", t=NQTOK // 128),
            )
            q16 = prep.tile([128, NQTOK], F16)
            nc.vector.tensor_copy(q16[:], q_nat[:])
            qT = prep.tile([128, NQTOK], F16)
            nc.sync.dma_start_transpose(
                qT[:].rearrange("p (t f) -> p t f", t=NQTOK // 128), q16[:])

            # d, one doc pair at a time so compute overlaps later loads
            dT = []
            for p in range(NPAIR):
                d_nat = dload_pool.tile([128, 2048], F32, tag="dnat",
                                        name="dnat")
                nc.sync.dma_start(
                    d_nat[:].rearrange("p (t d) -> p t d", t=16),
                    d_dram[2 * p:2 * p + 2].rearrange(
                        "c (t p) d -> p (c t) d", p=128),
                )
                d16 = dload_pool.tile([128, 2048], F16, tag="d16", name="d16")
                nc.vector.tensor_copy(d16[:], d_nat[:])
                dTp = prep.tile([128, 2048], F16, tag=f"dT{p}", name=f"dT{p}")
                nc.sync.dma_start_transpose(
                    dTp[:].rearrange("p (t f) -> p t f", t=16), d16[:])
                dT.append(dTp)

            # selector: sel[p, m] = 1 if p//32 == m (sums tokens per query)
            sel = prep.tile([128, 4], F16)
            nc.gpsimd.memset(sel[:], 0.0)
            for m in range(4):
                nc.gpsimd.memset(sel[32 * m:32 * (m + 1), m:m + 1], 1.0)

            # fp16: a 32-bit output AP would knock the batched
            # reduce_max off the DVE 2x/4x packed modes
            maxes = prep.tile([128, NSETS], F16)

            # ---- main loop: 16 query groups x 4 doc pairs ----
            for g in range(NG):
                n_m_pairs = 2 if (g % mper == 0) else 1
                n_b = NPAIR - n_m_pairs
                stgb = stgb_pool.tile([128, n_b * 2048], F16, tag="stgb",
                                      name="stgb") if n_b else None
                i_b = 0
                b_idx = []
                lhs = qT[:, bass.ts(g, 128)]
                for p in range(NPAIR):
                    idx = g * CL + 2 * p      # maxes column of doc 2p
                    pa = psum_pool.tile([128, 2048], F32, tag="pa", name="pa")
                    for cc in range(2):
                        base = cc * 1024
                        nc.tensor.matmul(
                            pa[:, base:base + 512], lhs,
                            dT[p][:, cc * 1024:cc * 1024 + 512],
                            start=True, stop=True)
                        nc.tensor.matmul(
                            pa[:, base + 512:base + 1024], lhs,
                            dT[p][:, cc * 1024 + 512:cc * 1024 + 1024],
                            start=True, stop=True)
                    if p < n_m_pairs:
                        # path M: DVE reduces both docs straight from PSUM
                        nc.vector.reduce_max(
                            maxes[:, idx:idx + 2],
                            pa[:].rearrange("p (c f) -> p c f", c=2),
                            axis=mybir.AxisListType.X)
                    else:
                        # path B: ACT stages to fp16; DVE reduces later
                        nc.scalar.copy(stgb[:, bass.ts(i_b, 2048)], pa[:])
                        b_idx.extend([idx, idx + 1])
                        i_b += 1
                if n_b:
                    assert b_idx == list(range(b_idx[0], b_idx[0] + 2 * n_b))
                    nc.vector.reduce_max(
                        maxes[:, b_idx[0]:b_idx[0] + 2 * n_b],
                        stgb[:].rearrange("p (s f) -> p s f", s=2 * n_b),
                        axis=mybir.AxisListType.X)

            # ---- reduce over the 32 tokens of each query ----
            sel_ps = psum_pool.tile([4, NSETS], F32, tag="pa", name="selps")
            nc.tensor.matmul(sel_ps[:], sel[:], maxes[:], start=True, stop=True)
            out_sb = prep.tile([4, NSETS], F32)
            nc.vector.tensor_copy(out_sb[:], sel_ps[:])
            nc.sync.dma_start(out_dram, out_sb[:])

    nc.finalize()
    return _patch_nc(nc)


_NC = None


def _get_nc():
    global _NC
    if _NC is None:
        _NC = build_nc()
    return _NC


def assemble_loss(outs, q):
    """Host tail: per-core [4, 128] blocks -> scores -> CE loss."""
    scores = np.zeros((B, B), np.float64)
    for k in range(NCORES):
        blk = np.asarray(outs[k], np.float64)  # [m=4, idx=g*8+c]
        for g in range(B // 4):
            for m in range(4):
                for c in range(CL):
                    scores[4 * g + m, CL * k + c] = blk[m, g * CL + c]
    if NORMALIZE_SCORES:
        q_len = (np.asarray(q)[:, :, 0] != 0).sum(axis=1).astype(np.float64)
        scores = scores / q_len[:, None]
    logits = scores / TEMPERATURE
    m = logits.max(axis=1, keepdims=True)
    logz = m[:, 0] + np.log(np.exp(logits - m).sum(axis=1))
    loss = -(np.diag(logits) - logz).mean()
    return np.float32(loss)


def kernel(query_embeddings, doc_embeddings):
    q = np.ascontiguousarray(np.asarray(query_embeddings, dtype=np.float32))
    d = np.ascontiguousarray(np.asarray(doc_embeddings, dtype=np.float32))
    nc = _get_nc()
    in_maps = [
        {"q": q, "d": np.ascontiguousarray(d[CL * k:CL * (k + 1)])}
        for k in range(NCORES)
    ]
    res = run_bass_kernel_spmd(nc, in_maps, core_ids=list(range(NCORES)))
    outs = [res.results[k]["out"] for k in range(NCORES)]
    return assemble_loss(outs, q)


# revision 18
# speedup vs baseline: 1.0423x; 1.0423x over previous
"""ColBERT MaxSim contrastive loss on 8 Trainium2 NeuronCores.

scores[b, c] = (1/q_len[b]) * sum_n max_s <q[b, n, :], d[c, s, :]>
loss = CE(scores / T, labels=arange(B)), mean reduction.

Sharding: data-parallel over the *doc* batch dim (columns of the score
matrix). Each core holds the full query set (1 MB) plus its 8-doc shard
(4 MB) instead of the all-gathered 32 MB doc tensor, computes its
(B_global, B_local) = (64, 8) score block fully on device (fp16 matmuls
at full PE rate + split max-reduction), and the host performs the final
gather + tiny 64x64 CE reduction (the same "host sums the partials"
tail as the standard contrastive sharding).

Device pipeline per core (HW-measured rates drive the design):
  1. q loads per 128-token chunk (contiguous 64 KB DMAs) so the first
     matmuls start within a few us; d loads per doc pair with 4-token
     grouping (2 KB descriptors; the token permutation inside each
     128-block is harmless because max over doc tokens is
     permutation-invariant). Both cast to fp16 and xbar-DMA-transposed
     into [D, token] layout (PE contracts over partitions).
  2. Per (query group g, doc): two fp16 matmuls -> [128, 1024] PSUM
     tile (4 rotating slots).
  3. Max-reduce drain. Measured: reduce_max is ~1 cyc/elem on every
     source (no fast uops), ACT copy ~1 elem/cyc, but fp16
     tensor_tensor hits the 2x packed mode (0.52 cyc/out). So:
       direct docs (~20%): DVE reduce_max straight off PSUM.
       staged docs: ACT copies PSUM -> fp16 SBUF; DVE folds with a
         3-level tensor_tensor(max) tree at 2x, then one small 1x
         reduce_max of the 128-wide remainders.
  4. A selector matmul sums the 32 token-maxes per query:
     out[4, 128] = sel.T @ maxes (fp16 to keep DVE modes available).
Host: out blocks -> scores (64, 64) -> q_len scaling -> CE loss.
"""

import json

import numpy as np

import concourse.bass as bass
import concourse.mybir as mybir
import concourse.tile as tile
from concourse.bass_utils import run_bass_kernel_spmd

B = 64          # queries (= docs, contrastive batch)
NQ = 32         # tokens per query
ND = 1024       # tokens per doc
D = 128         # embedding dim
NCORES = 8
CL = B // NCORES  # docs per core
TEMPERATURE = 0.02
NORMALIZE_SCORES = True

F32 = mybir.dt.float32
F16 = mybir.dt.float16

NG = (B * NQ) // 128        # 16 query groups of 4 queries
NPAIR = CL // 2             # 4 doc pairs per core
NSETS = NG * CL             # 128 (query group, doc) sets

# docs per query group drained directly by DVE (rest are ACT-staged);
# alternates N_DIR_EVEN/N_DIR_ODD to hit the DVE/ACT balance point.
N_DIR_EVEN = 2
N_DIR_ODD = 1


def _split_waits_json(bir_bytes: bytes) -> bytes:
    """Walrus in this toolchain rejects >1 sem-wait per instruction on the
    Tile end-of-kernel drain; split extra waits onto preceding Drains."""
    bir = json.loads(bir_bytes)
    for f in bir["functions"]:
        for blk in f["blocks"]:
            fixed = []
            for ins in blk["instructions"]:
                si = ins.get("sync_info") or {}
                waits = si.get("on_wait") or []
                if len(waits) > 1:
                    for i, w in enumerate(waits[:-1]):
                        fixed.append({
                            "debug": ins.get("debug", 0),
                            "engine": ins["engine"],
                            "ins": [],
                            "is_reset_sema": False,
                            "name": f'{ins["name"]}-wsplit{i}',
                            "opcode": "Drain",
                            "outs": [],
                            "sync_info": {"on_update": [], "on_wait": [w]},
                        })
                    si["on_wait"] = waits[-1:]
                    ins["sync_info"] = si
                fixed.append(ins)
            blk["instructions"] = fixed
    return json.dumps(bir).encode()


def _patch_nc(nc):
    orig = nc.to_json_bytes

    def patched(*a, **k):
        return _split_waits_json(orig(*a, **k))

    nc.to_json_bytes = patched
    return nc


def build_nc(n_dir_even=None, n_dir_odd=None):
    """Build the per-core Bass program (SPMD: every core runs this; only
    the data in its "d" shard differs)."""
    nde = N_DIR_EVEN if n_dir_even is None else n_dir_even
    ndo = N_DIR_ODD if n_dir_odd is None else n_dir_odd
    nc = bass.Bass("TRN2", target_bir_lowering=False, debug=False,
                   num_devices=NCORES)
    q_dram = nc.dram_tensor("q", [B, NQ, D], F32, kind="ExternalInput").ap()
    d_dram = nc.dram_tensor("d", [CL, ND, D], F32, kind="ExternalInput").ap()
    out_dram = nc.dram_tensor("out", [4, NSETS], F32, kind="ExternalOutput").ap()

    with tile.TileContext(nc) as tc:
        with (
            tc.tile_pool(name="prep", bufs=1) as prep,
            tc.tile_pool(name="qload", bufs=3) as qload_pool,
            tc.tile_pool(name="dload", bufs=2) as dload_pool,
            tc.tile_pool(name="stgb", bufs=2) as stgb_pool,
            tc.tile_pool(name="fold", bufs=2) as fold_pool,
            tc.tile_pool(name="mm", bufs=4, space="PSUM") as psum_pool,
        ):
            # ---- q: per-128-token chunk load -> cast -> transpose ----
            qT = prep.tile([128, NG * 128], F16)
            for t in range(NG):
                q_nat = qload_pool.tile([128, 128], F32, tag="qn", name="qn")
                # 4 queries = 128 tokens, fully contiguous 64 KB in DRAM
                nc.sync.dma_start(
                    q_nat[:],
                    q_dram[4 * t:4 * t + 4].rearrange("bb n d -> (bb n) d"))
                q16 = qload_pool.tile([128, 128], F16, tag="q6", name="q6")
                nc.vector.tensor_copy(q16[:], q_nat[:])
                nc.sync.dma_start_transpose(qT[:, bass.ts(t, 128)], q16[:])

            # ---- d: per doc pair, 2 KB descriptors (4-token groups;
            # the in-block token permutation is fine for max) ----
            dT = []
            for p in range(NPAIR):
                d_nat = dload_pool.tile([128, 2048], F32, tag="dnat",
                                        name="dnat")
                for c in range(2):
                    nc.sync.dma_start(
                        d_nat[:, c * 1024:(c + 1) * 1024].rearrange(
                            "p (t four d) -> p t four d", t=2, four=4),
                        d_dram[2 * p + c].rearrange(
                            "(t p four) d -> p t four d", p=128, four=4),
                    )
                d16 = dload_pool.tile([128, 2048], F16, tag="d16", name="d16")
                nc.vector.tensor_copy(d16[:], d_nat[:])
                dTp = prep.tile([128, 2048], F16, tag=f"dT{p}", name=f"dT{p}")
                nc.sync.dma_start_transpose(
                    dTp[:].rearrange("p (t f) -> p t f", t=16), d16[:])
                dT.append(dTp)

            # selector: sel[p, m] = 1 if p//32 == m (sums tokens per query)
            sel = prep.tile([128, 4], F16)
            nc.gpsimd.memset(sel[:], 0.0)
            for m in range(4):
                nc.gpsimd.memset(sel[32 * m:32 * (m + 1), m:m + 1], 1.0)

            # fp16 so DVE ops on it keep their packed modes
            maxes = prep.tile([128, NSETS], F16)

            # ---- main loop: 16 query groups x 8 docs ----
            for g in range(NG):
                n_dir = nde if g % 2 == 0 else ndo
                m = CL - n_dir
                stgb = stgb_pool.tile([128, m * 1024], F16, tag="stgb",
                                      name="stgb")
                st1 = fold_pool.tile([128, m * 512], F16, tag="st1",
                                     name="st1")
                st2 = fold_pool.tile([128, m * 256], F16, tag="st2",
                                     name="st2")
                st3 = fold_pool.tile([128, m * 128], F16, tag="st3",
                                     name="st3")
                i_b = 0
                lhs = qT[:, bass.ts(g, 128)]
                for doc in range(CL):
                    idx = g * CL + doc
                    pair, half = doc // 2, doc % 2
                    rhs = dT[pair][:, half * 1024:(half + 1) * 1024]
                    pa = psum_pool.tile([128, 1024], F32, tag="pa", name="pa")
                    nc.tensor.matmul(pa[:, 0:512], lhs, rhs[:, 0:512],
                                     start=True, stop=True)
                    nc.tensor.matmul(pa[:, 512:1024], lhs, rhs[:, 512:1024],
                                     start=True, stop=True)
                    if doc < n_dir:
                        nc.vector.reduce_max(maxes[:, idx:idx + 1], pa[:],
                                             axis=mybir.AxisListType.X)
                    else:
                        nc.scalar.copy(stgb[:, bass.ts(i_b, 1024)], pa[:])
                        i_b += 1
                # fp16 TT(max) fold tree at 2x, then one 1x reduce
                v0 = stgb[:].rearrange("p (s f) -> p s f", s=m)
                v1 = st1[:].rearrange("p (s f) -> p s f", s=m)
                v2 = st2[:].rearrange("p (s f) -> p s f", s=m)
                v3 = st3[:].rearrange("p (s f) -> p s f", s=m)
                nc.vector.tensor_max(out=v1, in0=v0[:, :, 0:512],
                                     in1=v0[:, :, 512:1024])
                nc.vector.tensor_max(out=v2, in0=v1[:, :, 0:256],
                                     in1=v1[:, :, 256:512])
                nc.vector.tensor_max(out=v3, in0=v2[:, :, 0:128],
                                     in1=v2[:, :, 128:256])
                base = g * CL + n_dir
                nc.vector.reduce_max(maxes[:, base:base + m], v3,
                                     axis=mybir.AxisListType.X)

            # ---- reduce over the 32 tokens of each query ----
            sel_ps = psum_pool.tile([4, NSETS], F32, tag="pa", name="selps")
            nc.tensor.matmul(sel_ps[:], sel[:], maxes[:], start=True, stop=True)
            out_sb = prep.tile([4, NSETS], F32)
            nc.vector.tensor_copy(out_sb[:], sel_ps[:])
            nc.sync.dma_start(out_dram, out_sb[:])

    nc.finalize()
    return _patch_nc(nc)


_NC = None


def _get_nc():
    global _NC
    if _NC is None:
        _NC = build_nc()
    return _NC


def assemble_loss(outs, q):
    """Host tail: per-core [4, 128] blocks -> scores -> CE loss."""
    scores = np.zeros((B, B), np.float64)
    for k in range(NCORES):
        blk = np.asarray(outs[k], np.float64)  # [m=4, idx=g*8+c]
        for g in range(B // 4):
            for m in range(4):
                for c in range(CL):
                    scores[4 * g + m, CL * k + c] = blk[m, g * CL + c]
    if NORMALIZE_SCORES:
        q_len = (np.asarray(q)[:, :, 0] != 0).sum(axis=1).astype(np.float64)
        scores = scores / q_len[:, None]
    logits = scores / TEMPERATURE
    m = logits.max(axis=1, keepdims=True)
    logz = m[:, 0] + np.log(np.exp(logits - m).sum(axis=1))
    loss = -(np.diag(logits) - logz).mean()
    return np.float32(loss)


def kernel(query_embeddings, doc_embeddings):
    q = np.ascontiguousarray(np.asarray(query_embeddings, dtype=np.float32))
    d = np.ascontiguousarray(np.asarray(doc_embeddings, dtype=np.float32))
    nc = _get_nc()
    in_maps = [
        {"q": q, "d": np.ascontiguousarray(d[CL * k:CL * (k + 1)])}
        for k in range(NCORES)
    ]
    res = run_bass_kernel_spmd(nc, in_maps, core_ids=list(range(NCORES)))
    outs = [res.results[k]["out"] for k in range(NCORES)]
    return assemble_loss(outs, q)
